# revision 30
# baseline (speedup 1.0000x reference)
"""Trainium2 Bass kernel for batched two-layer-MLP attention.

Reference semantics (per batch b):
    x  = sequence[:, b, :]                        # [S, D]
    K  = tanh(tanh(x @ Kw1.T) @ Kw2.T)
    Q  = tanh(tanh(x @ Qw1.T) @ Qw2.T)
    W  = softmax(K @ Q.T / sqrt(D), axis=-1)      # [S, S]
    out[:, b, :] = W @ x

Sharding: data-parallel over batch (B=8 -> 8 NeuronCores), weights replicated.
Compute in bf16 on the TensorEngine (fp32 PSUM accumulation); softmax in fp32.

Layout strategy per core:
  - xt = x.T  [D, S]  (bf16, host-pretransposed)  -> MLP moving operand
  - weights pre-transposed to [d_in, d_out] so they serve directly as lhsT
  - MLP outputs stay transposed: Kt, Qt in [D, S]
  - scores SC[s, t] = sum_d Kt[d,s] * Qt[d,t]: lhsT=Kt tile, rhs=Qt -> natural
  - softmax along free axis (t); exp's accum_out gives the row sums for free
  - each exp(SC) row-block is transposed with ONE xbar DMA-transpose (bf16)
    into lhsT layout for attended = Wt.T @ x with rhs = xn [S, D]
  - 1/rowsum is folded into the PSUM->SBUF copy of the output (per-partition
    activation scale), so the big W matrix is never normalized.

Precision plan (gate: rel_err < 2e-2; measured ~1.6e-2):
  - scores matmul in fp8 e4m3 DoubleRow (K=256 per instruction, 2x MAC rate)
  - last J8 j-blocks of each first MLP layer in fp8 DoubleRow (weights
    pre-scaled x32 on the host, compensated in the tanh activation scale)
  - everything else bf16 (fp16 measured 20% SLOWER than bf16 on the PE)

Scheduling tricks: a few HAM warmup matmuls bridge the engine-open to first-
DMA-landed window (the first real matmuls then absorb the cold-clock ramp);
x.T is host-packed n-chunk-contiguous so each chunk is ONE DMA instruction;
first-layer inputs are split across both HWDGE rings; phase B's scores PSUM
pool is opened early so its banks don't overlap phase A's; phase B runs in
super-blocks of 4 (4 blocks of fp8 scores, then their 4 bf16 attendeds) so
fp8<->bf16 PE mode switches are paid once per super-block and every
transpose has multiple blocks of latency cover; the final block's trailing
output chunks are halved to shorten the serial mul+DMA tail.
"""

import numpy as np
import ml_dtypes

import concourse.bacc as bacc
import concourse.tile as tile
from concourse import mybir
from concourse.bass_utils import run_bass_kernel_spmd

P = 128          # partitions
S = 2048         # sequence length
D = 1024         # model dim
B = 8            # batch (one per core)
ST = S // P      # 16 s-tiles
DT = D // P      # 8 d-tiles
NF = 512         # psum free width (one bank of fp32)
SN = S // NF     # 4 score free-chunks
DN = D // NF     # 2 output free-chunks
BF = mybir.dt.bfloat16     # fp16 measured 20% slower on the PE (259 vs 216 ns/mm)
F8 = mybir.dt.float8e4     # K/Q for the scores matmul (DoubleRow double-pump)
F32 = mybir.dt.float32
SCALE = 1.0 / np.sqrt(np.float32(D))
J8 = 3                     # trailing j-blocks of each MLP layer 1 done in fp8
W8SCALE = 32.0             # fp8 W1 pre-scale (keeps weights out of subnormals)

AX = mybir.AxisListType.X
AF = mybir.ActivationFunctionType


def build_nc():
    nc = bacc.Bacc("TRN2", target_bir_lowering=False)

    # x.T pre-packed on the host as [p, n, k, s-chunk] so each n-chunk loads
    # with ONE fully-contiguous DMA instruction (a strided [D,S] slice gets
    # split across queue slots and can stall the first psum group for ~6us)
    xc_d = nc.dram_tensor("xc", [P, SN, DT, NF], BF, kind="ExternalInput")
    xn_d = nc.dram_tensor("xn", [S, D], BF, kind="ExternalInput")
    # weights pre-arranged on the host to [p, j, k, c] so each j-block loads
    # with one partition-contiguous DMA (2KB/partition rows)
    WSHAPE = [P, DT, DT, P]
    wk1_d = nc.dram_tensor("wk1", WSHAPE, BF, kind="ExternalInput")
    wk2_d = nc.dram_tensor("wk2", WSHAPE, BF, kind="ExternalInput")
    wq1_d = nc.dram_tensor("wq1", WSHAPE, BF, kind="ExternalInput")
    wq2_d = nc.dram_tensor("wq2", WSHAPE, BF, kind="ExternalInput")
    # fp8 operands for the trailing J8 j-blocks of each first MLP layer:
    # x.T in e4m3 plus those W1 j-blocks pre-scaled x32 (compensated in the
    # tanh activation scale) so the weights clear e4m3's subnormal floor
    if J8:
        xt8_d = nc.dram_tensor("xt8", [D, S], F8, kind="ExternalInput")
        wk18_d = nc.dram_tensor("wk18", [P, J8, DT, P], F8, kind="ExternalInput")
        wq18_d = nc.dram_tensor("wq18", [P, J8, DT, P], F8, kind="ExternalInput")
    out_d = nc.dram_tensor("out", [S, D], F32, kind="ExternalOutput")

    from contextlib import ExitStack

    with tile.TileContext(nc) as tc, ExitStack() as ctx:
        # ---- persistent SBUF arrays (live across both phases) ----
        pers = ctx.enter_context(tc.tile_pool(name="pers", bufs=1))
        xn_sb = pers.tile([P, ST, D], BF)     # x normal: [t-part, t-tile, d]
        kt_sb = pers.tile([P, DT, S], F8)     # K.T: [d-part, d-tile, s]
        # Q.T split per n-chunk so phase B's first scores don't wait on the
        # whole tensor's last tanh
        qt_n = [pers.tile([P, DT, NF], F8, tag=f"qt{n}", name=f"qt{n}")
                for n in range(SN)]

        # scores PSUM pool opened before phase A so it gets banks disjoint
        # from the MLP pool - phase B's first matmul then has no released-pool
        # overlap dependency on phase A's tail
        psc = ctx.enter_context(tc.tile_pool(name="psum_sc", bufs=3, space="PSUM"))

        # ---- phase A: the four MLP layers ----
        with tc.tile_pool(name="phase_a", bufs=1) as pa, \
             tc.tile_pool(name="wpool", bufs=2) as wp, \
             tc.tile_pool(name="psum_mlp", bufs=4, space="PSUM") as pm:
            # x.T split into per-n-chunk tiles so the first psum row's matmuls
            # only wait on the 1MB slice they read, not the whole 4MB array
            xt_n = [pa.tile([P, DT, NF], BF, tag=f"xt{n}", name=f"xt{n}")
                    for n in range(SN)]
            xt8_sb = (pa.tile([P, DT, S], F8, tag="xt8", name="xt8")
                      if J8 else None)

            h1_sb = pa.tile([P, DT, S], BF)   # hidden activations (reused K/Q)

            # HAM warmup: throwaway matmuls bridging the ~1.3us between the
            # engines opening and the first input DMAs landing; the first few
            # real matmuls then run cold (~427ns) until the ~3.4us HAM window
            # passes, which costs less than idling through more warmups.
            warm_sb = pa.tile([P, NF], BF)
            nc.vector.memset(warm_sb, 0.0)
            warm_ps = pm.tile([P, NF], F32, tag="warm", bufs=1)
            NWARM = 6
            for i in range(NWARM):
                nc.tensor.matmul(warm_ps, warm_sb[:, 0:P], warm_sb,
                                 start=(i == 0), stop=(i == NWARM - 1))

            def mlp_layer(src, w_dram, dst, xdma=None, first=False,
                          w8_dram=None):
                # dst[j, s] = tanh(sum_k w[k, j].T @ src[k, s]) ; all transposed layout
                # one tile + one DMA per j-block so dep granularity is per-j.
                # When w8_dram is given (layer-1 calls), the last J8 j-blocks
                # run as fp8 e4m3 DoubleRow groups off xt8/w8 (x32 weight
                # scale compensated in the tanh input scale).
                nj_bf = DT - J8 if w8_dram is not None else DT
                w8 = (wp.tile([P, J8, DT, P], F8, tag="w8", name="w8")
                      if w8_dram is not None else None)
                if first:
                    # startup ordering: the DMA queues take ~2-3us to ramp
                    # and then run at a shared, modest rate, so the critical
                    # first-group inputs must be FIRST in their queues.
                    # SP ring: xc0 (k-halved so the first 4 matmuls only wait
                    # on 512KB), then w1/w3 + remaining x chunks. ACT ring:
                    # even-j weights. GPSIMD queue: the fp8 operands (not
                    # needed until the tail of the first n-sweep).
                    w_j = [wp.tile([P, DT, P], BF, tag=f"w{j}", name=f"w{j}")
                           for j in range(nj_bf)]
                    KH = DT // 2
                    nc.sync.dma_start(out=xt_n[0][:, 0:KH, :],
                                      in_=xc_d[:, 0, 0:KH, :])
                    nc.sync.dma_start(out=xt_n[0][:, KH:, :],
                                      in_=xc_d[:, 0, KH:, :])
                    for j in range(nj_bf):
                        ring = nc.scalar if j % 2 == 0 else nc.sync
                        ring.dma_start(out=w_j[j], in_=w_dram[:, j, :, :])
                    for n in range(1, SN):
                        nc.sync.dma_start(out=xt_n[n], in_=xc_d[:, n, :, :])
                    if w8 is not None:
                        x8_r = xt8_d.rearrange("(k p) s -> p k s", p=P)
                        nc.gpsimd.dma_start(out=xt8_sb, in_=x8_r)
                        nc.gpsimd.dma_start(out=w8, in_=w8_dram[:, :, :, :])
                else:
                    w_j = [wp.tile([P, DT, P], BF, tag=f"w{j}", name=f"w{j}")
                           for j in range(nj_bf)]
                    for j in range(nj_bf):
                        nc.sync.dma_start(out=w_j[j], in_=w_dram[:, j, :, :])
                    if w8 is not None:
                        nc.scalar.dma_start(out=w8, in_=w8_dram[:, :, :, :])
                    if xdma is not None:
                        xdma()

                def rhs_sl(n, k):
                    return (xt_n[n][:, k, :] if src is None
                            else src[:, k, n * NF:(n + 1) * NF])

                # fp8 j-blocks grouped contiguously (after the bf16 ones) so
                # the PE pays the fp8<->bf16 mode-switch cost once per layer,
                # not once per n-chunk
                loop = (([(j, n) for n in range(SN) for j in range(nj_bf)]
                         + [(j, n) for j in range(nj_bf, DT)
                            for n in range(SN)]) if first
                        else [(j, n) for j in range(DT) for n in range(SN)])
                for j, n in loop:
                    ps = pm.tile([P, NF], F32, tag="mlp")
                    if j < nj_bf:
                        for k in range(DT):
                            nc.tensor.matmul(
                                ps,
                                w_j[j][:, k, :],
                                rhs_sl(n, k),
                                start=(k == 0),
                                stop=(k == DT - 1),
                            )
                        scale = 1.0
                    else:
                        for k in range(0, DT, 2):
                            nc.tensor.matmul(
                                ps,
                                w8[:, j - nj_bf, k:k + 2, :],
                                xt8_sb[:, k:k + 2, n * NF:(n + 1) * NF],
                                start=(k == 0),
                                stop=(k == DT - 2),
                                perf_mode=mybir.MatmulPerfMode.DoubleRow,
                            )
                        scale = 1.0 / W8SCALE
                    dslice = (dst[n][:, j, :] if isinstance(dst, list)
                              else dst[:, j, n * NF:(n + 1) * NF])
                    nc.scalar.activation(out=dslice, in_=ps, func=AF.Tanh,
                                         scale=scale)

            def load_xn():
                xn_r = xn_d.rearrange("(t p) d -> p t d", p=P)
                for t in range(0, ST, 4):
                    nc.sync.dma_start(out=xn_sb[:, t:t + 4, :],
                                      in_=xn_r[:, t:t + 4, :])

            mlp_layer(None, wk1_d, h1_sb, first=True,
                      w8_dram=wk18_d if J8 else None)
            mlp_layer(h1_sb, wk2_d, kt_sb)
            mlp_layer(None, wq1_d, h1_sb, xdma=load_xn,
                      w8_dram=wq18_d if J8 else None)
            mlp_layer(h1_sb, wq2_d, qt_n)

        # ---- phase B: scores -> softmax -> transpose -> attended ----
        SB = 4   # super-block: 4 blocks of scores, then their 4 attendeds
        with tc.tile_pool(name="wexp", bufs=SB) as wexp_pool, \
             tc.tile_pool(name="wtT", bufs=SB + 1) as wtT_pool, \
             tc.tile_pool(name="sums", bufs=2 * SB + 2) as sums_pool, \
             tc.tile_pool(name="outst", bufs=2) as out_pool, \
             tc.tile_pool(name="psum_at", bufs=3, space="PSUM") as pat:

            def scores_softmax_transpose(i):
                """Row-block i of exp(scores) plus its reciprocal row sums,
                transposed into lhsT layout for the attended matmul."""
                wexp = wexp_pool.tile([P, S], BF, tag="wexp")
                sums = sums_pool.tile([P, SN], F32, tag="sums")
                for n in range(SN):
                    ps = psc.tile([P, NF], F32, tag="sc")
                    # fp8 DoubleRow: each matmul contracts a k-pair (K=256)
                    # at ~2 MACs/cell/cycle
                    for k in range(0, DT, 2):
                        nc.tensor.matmul(
                            ps,
                            kt_sb[:, k:k + 2, i * P:(i + 1) * P],
                            qt_n[n][:, k:k + 2, :],
                            start=(k == 0),
                            stop=(k == DT - 2),
                            perf_mode=mybir.MatmulPerfMode.DoubleRow,
                        )
                    # scores are bounded (|sc/32| < ~3): exp without max-shift
                    nc.scalar.activation(
                        out=wexp[:, n * NF:(n + 1) * NF],
                        in_=ps,
                        func=AF.Exp,
                        scale=float(SCALE),
                        accum_out=sums[:, n:n + 1],
                    )
                rcp = sums_pool.tile([P, 1], F32, tag="rcp")
                nc.vector.reduce_sum(rcp, sums, axis=AX)
                nc.vector.reciprocal(rcp, rcp)
                # one xbar transpose of the whole row-block:
                #   wtT[p, t, c] = wexp[c, t*128 + p]
                wtT = wtT_pool.tile([P, ST, P], BF, tag="wtT")
                nc.scalar.dma_start_transpose(out=wtT, in_=wexp)
                return wtT, rcp

            def attended(i, wtT, rcp, last=False):
                outst = out_pool.tile([P, D], F32, tag="outst")
                # on the final block, halve the trailing chunks so the
                # serial mul+DMA tail after the last matmul is shorter
                chunks = ([(0, NF), (NF, NF // 2), (NF + NF // 2, NF // 2)]
                          if last else [(0, NF), (NF, NF)])
                for off, w in chunks:
                    ps = pat.tile([P, w], F32, tag=f"at{w}", name=f"at{w}",
                                  bufs=3 if w == NF else 2)
                    for t in range(ST):
                        nc.tensor.matmul(
                            ps,
                            wtT[:, t, :],
                            xn_sb[:, t, off:off + w],
                            start=(t == 0),
                            stop=(t == ST - 1),
                        )
                    # fold the softmax normalization into the PSUM->SBUF copy
                    nc.scalar.mul(outst[:, off:off + w], ps, rcp)
                    nc.sync.dma_start(
                        out=out_d[i * P:(i + 1) * P, off:off + w],
                        in_=outst[:, off:off + w],
                    )

            # super-blocked: SB blocks of fp8 scores back-to-back, then their
            # SB bf16 attendeds - fp8<->bf16 mode switches drop from 2/block
            # to 2/super-block, and each transpose has SB-1 score blocks plus
            # the attended queue as latency cover
            for a in range(0, ST, SB):
                cur = [scores_softmax_transpose(i) for i in range(a, a + SB)]
                for i in range(a, a + SB):
                    attended(i, *cur[i - a], last=(i == ST - 1))

    nc.compile()
    return nc


_NC = None


def _get_nc():
    global _NC
    if _NC is None:
        _NC = build_nc()
    return _NC


def _prep_w(w, dt=None, scale=1.0):
    """[d_out, d_in] f32 -> [p, j, k, c] of w.T (k,p index d_in; j,c d_out)."""
    wt = (np.asarray(w).T * scale).reshape(DT, P, DT, P).transpose(1, 2, 0, 3)
    return np.ascontiguousarray(wt).astype(dt or ml_dtypes.bfloat16)


def make_in_maps(sequence, Kw1, Kw2, Qw1, Qw2):
    bf16 = ml_dtypes.bfloat16
    seq = np.ascontiguousarray(np.transpose(np.asarray(sequence), (1, 0, 2)))  # [B, S, D]
    ws = {"wk1": _prep_w(Kw1), "wk2": _prep_w(Kw2),
          "wq1": _prep_w(Qw1), "wq2": _prep_w(Qw2)}
    if J8:
        e4 = ml_dtypes.float8_e4m3
        ws["wk18"] = np.ascontiguousarray(
            _prep_w(Kw1, e4, W8SCALE)[:, DT - J8:, :, :])
        ws["wq18"] = np.ascontiguousarray(
            _prep_w(Qw1, e4, W8SCALE)[:, DT - J8:, :, :])
    in_maps = []
    for b in range(B):
        xb = seq[b]
        xtf = np.ascontiguousarray(xb.T)                    # [D, S] fp32
        xc = np.ascontiguousarray(                          # [P, SN, DT, NF]
            xtf.reshape(DT, P, SN, NF).transpose(1, 2, 0, 3))
        m = {"xn": xb.astype(bf16), "xc": xc.astype(bf16)}
        if J8:
            m["xt8"] = xtf.astype(ml_dtypes.float8_e4m3)
        m.update(ws)
        in_maps.append(m)
    return in_maps


def kernel(sequence, Kw1, Kw2, Qw1, Qw2):
    nc = _get_nc()
    in_maps = make_in_maps(sequence, Kw1, Kw2, Qw1, Qw2)
    res = run_bass_kernel_spmd(nc, in_maps, core_ids=list(range(B)))
    out = np.stack([res.results[b]["out"] for b in range(B)], axis=1)
    return out.astype(np.float32)



# revision 31
# speedup vs baseline: 1.2104x; 1.2104x over previous
"""Trainium2 Bass kernel for batched two-layer-MLP attention.

Reference semantics (per batch b):
    x  = sequence[:, b, :]                        # [S, D]
    K  = tanh(tanh(x @ Kw1.T) @ Kw2.T)
    Q  = tanh(tanh(x @ Qw1.T) @ Qw2.T)
    W  = softmax(K @ Q.T / sqrt(D), axis=-1)      # [S, S]
    out[:, b, :] = W @ x

Sharding: data-parallel over batch (B=8 -> 8 NeuronCores), weights replicated.
Compute in bf16 on the TensorEngine (fp32 PSUM accumulation); softmax in fp32.

Layout strategy per core:
  - xt = x.T  [D, S]  (bf16, host-pretransposed)  -> MLP moving operand
  - weights pre-transposed to [d_in, d_out] so they serve directly as lhsT
  - MLP outputs stay transposed: Kt, Qt in [D, S]
  - scores SC[s, t] = sum_d Kt[d,s] * Qt[d,t]: lhsT=Kt tile, rhs=Qt -> natural
  - softmax along free axis (t); exp's accum_out gives the row sums for free
  - each exp(SC) row-block is transposed with ONE xbar DMA-transpose (bf16)
    into lhsT layout for attended = Wt.T @ x with rhs = xn [S, D]
  - 1/rowsum is folded into the PSUM->SBUF copy of the output (per-partition
    activation scale), so the big W matrix is never normalized.

Precision plan (gate: rel_err < 2e-2; measured ~1.6e-2):
  - scores matmul in fp8 e4m3 DoubleRow (K=256 per instruction, 2x MAC rate)
  - last J8 j-blocks of each first MLP layer in fp8 DoubleRow (weights
    pre-scaled x32 on the host, compensated in the tanh activation scale)
  - everything else bf16 (fp16 measured 20% SLOWER than bf16 on the PE)

Scheduling tricks: a few HAM warmup matmuls bridge the engine-open to first-
DMA-landed window (the first real matmuls then absorb the cold-clock ramp);
x.T is host-packed n-chunk-contiguous so each chunk is ONE DMA instruction;
first-layer inputs are split across both HWDGE rings; phase B's scores PSUM
pool is opened early so its banks don't overlap phase A's; phase B runs in
super-blocks of 4 (4 blocks of fp8 scores, then their 4 bf16 attendeds) so
fp8<->bf16 PE mode switches are paid once per super-block and every
transpose has multiple blocks of latency cover; the final block's trailing
output chunks are halved to shorten the serial mul+DMA tail.
"""

import numpy as np
import ml_dtypes

import concourse.bacc as bacc
import concourse.tile as tile
from concourse import mybir
from concourse.bass_utils import run_bass_kernel_spmd

P = 128          # partitions
S = 2048         # sequence length
D = 1024         # model dim
B = 8            # batch (one per core)
ST = S // P      # 16 s-tiles
DT = D // P      # 8 d-tiles
NF = 512         # psum free width (one bank of fp32)
SN = S // NF     # 4 score free-chunks
DN = D // NF     # 2 output free-chunks
BF = mybir.dt.bfloat16     # fp16 measured 20% slower on the PE (259 vs 216 ns/mm)
F8 = mybir.dt.float8e4     # K/Q for the scores matmul (DoubleRow double-pump)
F32 = mybir.dt.float32
SCALE = 1.0 / np.sqrt(np.float32(D))
J8 = 3                     # trailing j-blocks of each MLP layer 1 done in fp8
W8SCALE = 32.0             # fp8 W1 pre-scale (keeps weights out of subnormals)

AX = mybir.AxisListType.X
AF = mybir.ActivationFunctionType


def build_nc():
    nc = bacc.Bacc("TRN2", target_bir_lowering=False)

    # x.T pre-packed on the host as [p, n, k, s-chunk] so each n-chunk loads
    # with ONE fully-contiguous DMA instruction (a strided [D,S] slice gets
    # split across queue slots and can stall the first psum group for ~6us)
    xc_d = nc.dram_tensor("xc", [P, SN, DT, NF], BF, kind="ExternalInput")
    xn_d = nc.dram_tensor("xn", [S, D], BF, kind="ExternalInput")
    # weights pre-arranged on the host to [p, j, k, c] so each j-block loads
    # with one partition-contiguous DMA (2KB/partition rows)
    WSHAPE = [P, DT, DT, P]
    wk1_d = nc.dram_tensor("wk1", WSHAPE, BF, kind="ExternalInput")
    wk2_d = nc.dram_tensor("wk2", WSHAPE, BF, kind="ExternalInput")
    wq1_d = nc.dram_tensor("wq1", WSHAPE, BF, kind="ExternalInput")
    wq2_d = nc.dram_tensor("wq2", WSHAPE, BF, kind="ExternalInput")
    # fp8 operands for the trailing J8 j-blocks of each first MLP layer:
    # x.T in e4m3 plus those W1 j-blocks pre-scaled x32 (compensated in the
    # tanh activation scale) so the weights clear e4m3's subnormal floor
    if J8:
        xt8_d = nc.dram_tensor("xt8", [D, S], F8, kind="ExternalInput")
        wk18_d = nc.dram_tensor("wk18", [P, J8, DT, P], F8, kind="ExternalInput")
        wq18_d = nc.dram_tensor("wq18", [P, J8, DT, P], F8, kind="ExternalInput")
    out_d = nc.dram_tensor("out", [S, D], F32, kind="ExternalOutput")

    from contextlib import ExitStack

    with tile.TileContext(nc) as tc, ExitStack() as ctx:
        # ---- persistent SBUF arrays (live across both phases) ----
        pers = ctx.enter_context(tc.tile_pool(name="pers", bufs=1))
        xn_sb = pers.tile([P, ST, D], BF)     # x normal: [t-part, t-tile, d]
        kt_sb = pers.tile([P, DT, S], F8)     # K.T: [d-part, d-tile, s]
        # Q.T split per n-chunk so phase B's first scores don't wait on the
        # whole tensor's last tanh
        qt_n = [pers.tile([P, DT, NF], F8, tag=f"qt{n}", name=f"qt{n}")
                for n in range(SN)]

        # scores PSUM pool opened before phase A so it gets banks disjoint
        # from the MLP pool - phase B's first matmul then has no released-pool
        # overlap dependency on phase A's tail
        psc = ctx.enter_context(tc.tile_pool(name="psum_sc", bufs=3, space="PSUM"))

        # ---- phase A: the four MLP layers ----
        with tc.tile_pool(name="phase_a", bufs=1) as pa, \
             tc.tile_pool(name="wpool", bufs=2) as wp, \
             tc.tile_pool(name="psum_mlp", bufs=4, space="PSUM") as pm:
            # x.T split into per-n-chunk tiles so the first psum row's matmuls
            # only wait on the 1MB slice they read, not the whole 4MB array
            xt_n = [pa.tile([P, DT, NF], BF, tag=f"xt{n}", name=f"xt{n}")
                    for n in range(SN)]
            xt8_sb = (pa.tile([P, DT, S], F8, tag="xt8", name="xt8")
                      if J8 else None)

            h1_sb = pa.tile([P, DT, S], BF)   # hidden activations (reused K/Q)

            # HAM warmup: throwaway matmuls bridging the ~1.3us between the
            # engines opening and the first input DMAs landing; the first few
            # real matmuls then run cold (~427ns) until the ~3.4us HAM window
            # passes, which costs less than idling through more warmups.
            warm_sb = pa.tile([P, NF], BF)
            nc.vector.memset(warm_sb, 0.0)
            warm_ps = pm.tile([P, NF], F32, tag="warm", bufs=1)
            NWARM = 6
            for i in range(NWARM):
                nc.tensor.matmul(warm_ps, warm_sb[:, 0:P], warm_sb,
                                 start=(i == 0), stop=(i == NWARM - 1))

            def mlp_layer(src, w_dram, dst, xdma=None, first=False,
                          w8_dram=None):
                # dst[j, s] = tanh(sum_k w[k, j].T @ src[k, s]) ; all transposed layout
                # one tile + one DMA per j-block so dep granularity is per-j.
                # When w8_dram is given (layer-1 calls), the last J8 j-blocks
                # run as fp8 e4m3 DoubleRow groups off xt8/w8 (x32 weight
                # scale compensated in the tanh input scale).
                nj_bf = DT - J8 if w8_dram is not None else DT
                w8 = (wp.tile([P, J8, DT, P], F8, tag="w8", name="w8")
                      if w8_dram is not None else None)
                if first:
                    # startup ordering: the DMA queues take ~2-3us to ramp
                    # and then run at a shared, modest rate, so the critical
                    # first-group inputs must be FIRST in their queues.
                    # SP ring: xc0 (k-halved so the first 4 matmuls only wait
                    # on 512KB), then w1/w3 + remaining x chunks. ACT ring:
                    # even-j weights. GPSIMD queue: the fp8 operands (not
                    # needed until the tail of the first n-sweep).
                    w_j = [wp.tile([P, DT, P], BF, tag=f"w{j}", name=f"w{j}")
                           for j in range(nj_bf)]
                    KH = DT // 2
                    nc.sync.dma_start(out=xt_n[0][:, 0:KH, :],
                                      in_=xc_d[:, 0, 0:KH, :])
                    nc.sync.dma_start(out=xt_n[0][:, KH:, :],
                                      in_=xc_d[:, 0, KH:, :])
                    for j in range(nj_bf):
                        ring = nc.scalar if j % 2 == 0 else nc.sync
                        ring.dma_start(out=w_j[j], in_=w_dram[:, j, :, :])
                    for n in range(1, SN):
                        nc.sync.dma_start(out=xt_n[n], in_=xc_d[:, n, :, :])
                    if w8 is not None:
                        x8_r = xt8_d.rearrange("(k p) s -> p k s", p=P)
                        nc.scalar.dma_start(out=xt8_sb, in_=x8_r)
                        nc.scalar.dma_start(out=w8, in_=w8_dram[:, :, :, :])
                else:
                    w_j = [wp.tile([P, DT, P], BF, tag=f"w{j}", name=f"w{j}")
                           for j in range(nj_bf)]
                    for j in range(nj_bf):
                        nc.sync.dma_start(out=w_j[j], in_=w_dram[:, j, :, :])
                    if w8 is not None:
                        nc.scalar.dma_start(out=w8, in_=w8_dram[:, :, :, :])
                    if xdma is not None:
                        xdma()

                def rhs_sl(n, k):
                    return (xt_n[n][:, k, :] if src is None
                            else src[:, k, n * NF:(n + 1) * NF])

                # fp8 j-blocks grouped contiguously (after the bf16 ones) so
                # the PE pays the fp8<->bf16 mode-switch cost once per layer,
                # not once per n-chunk
                loop = (([(j, n) for n in range(SN) for j in range(nj_bf)]
                         + [(j, n) for j in range(nj_bf, DT)
                            for n in range(SN)]) if first
                        else [(j, n) for j in range(DT) for n in range(SN)])
                for j, n in loop:
                    ps = pm.tile([P, NF], F32, tag="mlp")
                    if j < nj_bf:
                        for k in range(DT):
                            nc.tensor.matmul(
                                ps,
                                w_j[j][:, k, :],
                                rhs_sl(n, k),
                                start=(k == 0),
                                stop=(k == DT - 1),
                            )
                        scale = 1.0
                    else:
                        for k in range(0, DT, 2):
                            nc.tensor.matmul(
                                ps,
                                w8[:, j - nj_bf, k:k + 2, :],
                                xt8_sb[:, k:k + 2, n * NF:(n + 1) * NF],
                                start=(k == 0),
                                stop=(k == DT - 2),
                                perf_mode=mybir.MatmulPerfMode.DoubleRow,
                            )
                        scale = 1.0 / W8SCALE
                    dslice = (dst[n][:, j, :] if isinstance(dst, list)
                              else dst[:, j, n * NF:(n + 1) * NF])
                    nc.scalar.activation(out=dslice, in_=ps, func=AF.Tanh,
                                         scale=scale)

            def load_xn():
                xn_r = xn_d.rearrange("(t p) d -> p t d", p=P)
                for t in range(0, ST, 4):
                    nc.sync.dma_start(out=xn_sb[:, t:t + 4, :],
                                      in_=xn_r[:, t:t + 4, :])

            mlp_layer(None, wk1_d, h1_sb, first=True,
                      w8_dram=wk18_d if J8 else None)
            mlp_layer(h1_sb, wk2_d, kt_sb)
            mlp_layer(None, wq1_d, h1_sb, xdma=load_xn,
                      w8_dram=wq18_d if J8 else None)
            mlp_layer(h1_sb, wq2_d, qt_n)

        # ---- phase B: scores -> softmax -> transpose -> attended ----
        SB = 4   # super-block: 4 blocks of scores, then their 4 attendeds
        with tc.tile_pool(name="wexp", bufs=SB) as wexp_pool, \
             tc.tile_pool(name="wtT", bufs=SB + 1) as wtT_pool, \
             tc.tile_pool(name="sums", bufs=2 * SB + 2) as sums_pool, \
             tc.tile_pool(name="outst", bufs=2) as out_pool, \
             tc.tile_pool(name="psum_at", bufs=3, space="PSUM") as pat:

            def scores_softmax_transpose(i):
                """Row-block i of exp(scores) plus its reciprocal row sums,
                transposed into lhsT layout for the attended matmul."""
                wexp = wexp_pool.tile([P, S], BF, tag="wexp")
                sums = sums_pool.tile([P, SN], F32, tag="sums")
                for n in range(SN):
                    ps = psc.tile([P, NF], F32, tag="sc")
                    # fp8 DoubleRow: each matmul contracts a k-pair (K=256)
                    # at ~2 MACs/cell/cycle
                    for k in range(0, DT, 2):
                        nc.tensor.matmul(
                            ps,
                            kt_sb[:, k:k + 2, i * P:(i + 1) * P],
                            qt_n[n][:, k:k + 2, :],
                            start=(k == 0),
                            stop=(k == DT - 2),
                            perf_mode=mybir.MatmulPerfMode.DoubleRow,
                        )
                    # scores are bounded (|sc/32| < ~3): exp without max-shift
                    nc.scalar.activation(
                        out=wexp[:, n * NF:(n + 1) * NF],
                        in_=ps,
                        func=AF.Exp,
                        scale=float(SCALE),
                        accum_out=sums[:, n:n + 1],
                    )
                rcp = sums_pool.tile([P, 1], F32, tag="rcp")
                nc.vector.reduce_sum(rcp, sums, axis=AX)
                nc.vector.reciprocal(rcp, rcp)
                # one xbar transpose of the whole row-block:
                #   wtT[p, t, c] = wexp[c, t*128 + p]
                wtT = wtT_pool.tile([P, ST, P], BF, tag="wtT")
                nc.scalar.dma_start_transpose(out=wtT, in_=wexp)
                return wtT, rcp

            def attended(i, wtT, rcp, last=False):
                outst = out_pool.tile([P, D], F32, tag="outst")
                # on the final block, halve the trailing chunks so the
                # serial mul+DMA tail after the last matmul is shorter
                chunks = ([(0, NF), (NF, NF // 2), (NF + NF // 2, NF // 2)]
                          if last else [(0, NF), (NF, NF)])
                for off, w in chunks:
                    ps = pat.tile([P, w], F32, tag=f"at{w}", name=f"at{w}",
                                  bufs=3 if w == NF else 2)
                    for t in range(ST):
                        nc.tensor.matmul(
                            ps,
                            wtT[:, t, :],
                            xn_sb[:, t, off:off + w],
                            start=(t == 0),
                            stop=(t == ST - 1),
                        )
                    # fold the softmax normalization into the PSUM->SBUF copy
                    nc.scalar.mul(outst[:, off:off + w], ps, rcp)
                    nc.sync.dma_start(
                        out=out_d[i * P:(i + 1) * P, off:off + w],
                        in_=outst[:, off:off + w],
                    )

            # super-blocked: SB blocks of fp8 scores back-to-back, then their
            # SB bf16 attendeds - fp8<->bf16 mode switches drop from 2/block
            # to 2/super-block, and each transpose has SB-1 score blocks plus
            # the attended queue as latency cover
            for a in range(0, ST, SB):
                cur = [scores_softmax_transpose(i) for i in range(a, a + SB)]
                for i in range(a, a + SB):
                    attended(i, *cur[i - a], last=(i == ST - 1))

    nc.compile()
    return nc


_NC = None


def _get_nc():
    global _NC
    if _NC is None:
        _NC = build_nc()
    return _NC


def _prep_w(w, dt=None, scale=1.0):
    """[d_out, d_in] f32 -> [p, j, k, c] of w.T (k,p index d_in; j,c d_out)."""
    wt = (np.asarray(w).T * scale).reshape(DT, P, DT, P).transpose(1, 2, 0, 3)
    return np.ascontiguousarray(wt).astype(dt or ml_dtypes.bfloat16)


def make_in_maps(sequence, Kw1, Kw2, Qw1, Qw2):
    bf16 = ml_dtypes.bfloat16
    seq = np.ascontiguousarray(np.transpose(np.asarray(sequence), (1, 0, 2)))  # [B, S, D]
    ws = {"wk1": _prep_w(Kw1), "wk2": _prep_w(Kw2),
          "wq1": _prep_w(Qw1), "wq2": _prep_w(Qw2)}
    if J8:
        e4 = ml_dtypes.float8_e4m3
        ws["wk18"] = np.ascontiguousarray(
            _prep_w(Kw1, e4, W8SCALE)[:, DT - J8:, :, :])
        ws["wq18"] = np.ascontiguousarray(
            _prep_w(Qw1, e4, W8SCALE)[:, DT - J8:, :, :])
    in_maps = []
    for b in range(B):
        xb = seq[b]
        xtf = np.ascontiguousarray(xb.T)                    # [D, S] fp32
        xc = np.ascontiguousarray(                          # [P, SN, DT, NF]
            xtf.reshape(DT, P, SN, NF).transpose(1, 2, 0, 3))
        m = {"xn": xb.astype(bf16), "xc": xc.astype(bf16)}
        if J8:
            m["xt8"] = xtf.astype(ml_dtypes.float8_e4m3)
        m.update(ws)
        in_maps.append(m)
    return in_maps


def kernel(sequence, Kw1, Kw2, Qw1, Qw2):
    nc = _get_nc()
    in_maps = make_in_maps(sequence, Kw1, Kw2, Qw1, Qw2)
    res = run_bass_kernel_spmd(nc, in_maps, core_ids=list(range(B)))
    out = np.stack([res.results[b]["out"] for b in range(B)], axis=1)
    return out.astype(np.float32)



# revision 34
# speedup vs baseline: 1.2142x; 1.0032x over previous
"""Trainium2 Bass kernel for batched two-layer-MLP attention.

Reference semantics (per batch b):
    x  = sequence[:, b, :]                        # [S, D]
    K  = tanh(tanh(x @ Kw1.T) @ Kw2.T)
    Q  = tanh(tanh(x @ Qw1.T) @ Qw2.T)
    W  = softmax(K @ Q.T / sqrt(D), axis=-1)      # [S, S]
    out[:, b, :] = W @ x

Sharding: data-parallel over batch (B=8 -> 8 NeuronCores), weights replicated.
Compute in bf16 on the TensorEngine (fp32 PSUM accumulation); softmax in fp32.

Layout strategy per core:
  - xt = x.T  [D, S]  (bf16, host-pretransposed)  -> MLP moving operand
  - weights pre-transposed to [d_in, d_out] so they serve directly as lhsT
  - MLP outputs stay transposed: Kt, Qt in [D, S]
  - scores SC[s, t] = sum_d Kt[d,s] * Qt[d,t]: lhsT=Kt tile, rhs=Qt -> natural
  - softmax along free axis (t); exp's accum_out gives the row sums for free
  - each exp(SC) row-block is transposed with ONE xbar DMA-transpose (bf16)
    into lhsT layout for attended = Wt.T @ x with rhs = xn [S, D]
  - 1/rowsum is folded into the PSUM->SBUF copy of the output (per-partition
    activation scale), so the big W matrix is never normalized.

Precision plan (gate: rel_err < 2e-2; measured ~1.6e-2):
  - scores matmul in fp8 e4m3 DoubleRow (K=256 per instruction, 2x MAC rate)
  - last J8 j-blocks of each first MLP layer in fp8 DoubleRow (weights
    pre-scaled x32 on the host, compensated in the tanh activation scale)
  - everything else bf16 (fp16 measured 20% SLOWER than bf16 on the PE)

Scheduling tricks: a few HAM warmup matmuls bridge the engine-open to first-
DMA-landed window (the first real matmuls then absorb the cold-clock ramp);
x.T is host-packed n-chunk-contiguous so each chunk is ONE DMA instruction;
first-layer inputs are split across both HWDGE rings; phase B's scores PSUM
pool is opened early so its banks don't overlap phase A's; phase B runs in
super-blocks of 4 (4 blocks of fp8 scores, then their 4 bf16 attendeds) so
fp8<->bf16 PE mode switches are paid once per super-block and every
transpose has multiple blocks of latency cover; the final block's trailing
output chunks are halved to shorten the serial mul+DMA tail.
"""

import numpy as np
import ml_dtypes

import concourse.bacc as bacc
import concourse.tile as tile
from concourse import mybir
from concourse.bass_utils import run_bass_kernel_spmd

P = 128          # partitions
S = 2048         # sequence length
D = 1024         # model dim
B = 8            # batch (one per core)
ST = S // P      # 16 s-tiles
DT = D // P      # 8 d-tiles
NF = 512         # psum free width (one bank of fp32)
SN = S // NF     # 4 score free-chunks
DN = D // NF     # 2 output free-chunks
BF = mybir.dt.bfloat16     # fp16 measured 20% slower on the PE (259 vs 216 ns/mm)
F8 = mybir.dt.float8e4     # K/Q for the scores matmul (DoubleRow double-pump)
F32 = mybir.dt.float32
SCALE = 1.0 / np.sqrt(np.float32(D))
J8 = 3                     # trailing j-blocks of each MLP layer 1 done in fp8
W8SCALE = 32.0             # fp8 W1 pre-scale (keeps weights out of subnormals)

AX = mybir.AxisListType.X
AF = mybir.ActivationFunctionType


def build_nc():
    nc = bacc.Bacc("TRN2", target_bir_lowering=False)

    # x.T pre-packed on the host as [p, n, k, s-chunk] so each n-chunk loads
    # with ONE fully-contiguous DMA instruction (a strided [D,S] slice gets
    # split across queue slots and can stall the first psum group for ~6us)
    xc_d = nc.dram_tensor("xc", [P, SN, DT, NF], BF, kind="ExternalInput")
    xn_d = nc.dram_tensor("xn", [S, D], BF, kind="ExternalInput")
    # weights pre-arranged on the host to [p, j, k, c] so each j-block loads
    # with one partition-contiguous DMA (2KB/partition rows)
    WSHAPE = [P, DT, DT, P]
    wk1_d = nc.dram_tensor("wk1", WSHAPE, BF, kind="ExternalInput")
    wk2_d = nc.dram_tensor("wk2", WSHAPE, BF, kind="ExternalInput")
    wq1_d = nc.dram_tensor("wq1", WSHAPE, BF, kind="ExternalInput")
    wq2_d = nc.dram_tensor("wq2", WSHAPE, BF, kind="ExternalInput")
    # fp8 operands for the trailing J8 j-blocks of each first MLP layer:
    # x.T in e4m3 plus those W1 j-blocks pre-scaled x32 (compensated in the
    # tanh activation scale) so the weights clear e4m3's subnormal floor
    if J8:
        xt8_d = nc.dram_tensor("xt8", [D, S], F8, kind="ExternalInput")
        wk18_d = nc.dram_tensor("wk18", [P, J8, DT, P], F8, kind="ExternalInput")
        wq18_d = nc.dram_tensor("wq18", [P, J8, DT, P], F8, kind="ExternalInput")
    out_d = nc.dram_tensor("out", [S, D], F32, kind="ExternalOutput")

    from contextlib import ExitStack

    with tile.TileContext(nc) as tc, ExitStack() as ctx:
        # ---- persistent SBUF arrays (live across both phases) ----
        pers = ctx.enter_context(tc.tile_pool(name="pers", bufs=1))
        xn_sb = pers.tile([P, ST, D], BF)     # x normal: [t-part, t-tile, d]
        kt_sb = pers.tile([P, DT, S], F8)     # K.T: [d-part, d-tile, s]
        # Q.T split per n-chunk so phase B's first scores don't wait on the
        # whole tensor's last tanh
        qt_n = [pers.tile([P, DT, NF], F8, tag=f"qt{n}", name=f"qt{n}")
                for n in range(SN)]

        # scores PSUM pool opened before phase A so it gets banks disjoint
        # from the MLP pool - phase B's first matmul then has no released-pool
        # overlap dependency on phase A's tail
        psc = ctx.enter_context(tc.tile_pool(name="psum_sc", bufs=3, space="PSUM"))

        # ---- phase A: the four MLP layers ----
        with tc.tile_pool(name="phase_a", bufs=1) as pa, \
             tc.tile_pool(name="wpool", bufs=2) as wp, \
             tc.tile_pool(name="psum_mlp", bufs=4, space="PSUM") as pm:
            # x.T split into per-n-chunk tiles so the first psum row's matmuls
            # only wait on the 1MB slice they read, not the whole 4MB array
            xt_n = [pa.tile([P, DT, NF], BF, tag=f"xt{n}", name=f"xt{n}")
                    for n in range(SN)]
            xt8_sb = (pa.tile([P, DT, S], F8, tag="xt8", name="xt8")
                      if J8 else None)

            h1_sb = pa.tile([P, DT, S], BF)   # hidden activations (reused K/Q)

            # HAM warmup: throwaway matmuls bridging the ~1.3us between the
            # engines opening and the first input DMAs landing; the first few
            # real matmuls then run cold (~427ns) until the ~3.4us HAM window
            # passes, which costs less than idling through more warmups.
            warm_sb = pa.tile([P, NF], BF)
            nc.vector.memset(warm_sb, 0.0)
            warm_ps = pm.tile([P, NF], F32, tag="warm", bufs=1)
            NWARM = 6
            for i in range(NWARM):
                nc.tensor.matmul(warm_ps, warm_sb[:, 0:P], warm_sb,
                                 start=(i == 0), stop=(i == NWARM - 1))

            def mlp_layer(src, w_dram, dst, xdma=None, first=False,
                          w8_dram=None):
                # dst[j, s] = tanh(sum_k w[k, j].T @ src[k, s]) ; all transposed layout
                # one tile + one DMA per j-block so dep granularity is per-j.
                # When w8_dram is given (layer-1 calls), the last J8 j-blocks
                # run as fp8 e4m3 DoubleRow groups off xt8/w8 (x32 weight
                # scale compensated in the tanh input scale).
                nj_bf = DT - J8 if w8_dram is not None else DT
                w8 = (wp.tile([P, J8, DT, P], F8, tag="w8", name="w8")
                      if w8_dram is not None else None)
                if first:
                    # startup ordering: the DMA queues take ~2-3us to ramp
                    # and then run at a shared, modest rate, so the critical
                    # first-group inputs must be FIRST in their queues.
                    # SP ring: xc0 (k-halved so the first 4 matmuls only wait
                    # on 512KB), then w1/w3 + remaining x chunks. ACT ring:
                    # even-j weights. GPSIMD queue: the fp8 operands (not
                    # needed until the tail of the first n-sweep).
                    w_j = [wp.tile([P, DT, P], BF, tag=f"w{j}", name=f"w{j}")
                           for j in range(nj_bf)]
                    KH = DT // 2
                    nc.sync.dma_start(out=xt_n[0][:, 0:KH, :],
                                      in_=xc_d[:, 0, 0:KH, :])
                    nc.sync.dma_start(out=xt_n[0][:, KH:, :],
                                      in_=xc_d[:, 0, KH:, :])
                    for j in range(nj_bf):
                        ring = nc.scalar if j % 2 == 0 else nc.sync
                        ring.dma_start(out=w_j[j], in_=w_dram[:, j, :, :])
                    for n in range(1, SN):
                        nc.sync.dma_start(out=xt_n[n], in_=xc_d[:, n, :, :])
                    if w8 is not None:
                        x8_r = xt8_d.rearrange("(k p) s -> p k s", p=P)
                        nc.scalar.dma_start(out=xt8_sb, in_=x8_r)
                        nc.scalar.dma_start(out=w8, in_=w8_dram[:, :, :, :])
                else:
                    w_j = [wp.tile([P, DT, P], BF, tag=f"w{j}", name=f"w{j}")
                           for j in range(nj_bf)]
                    for j in range(nj_bf):
                        nc.sync.dma_start(out=w_j[j], in_=w_dram[:, j, :, :])
                    if w8 is not None:
                        nc.scalar.dma_start(out=w8, in_=w8_dram[:, :, :, :])
                    if xdma is not None:
                        xdma()

                def rhs_sl(n, k):
                    return (xt_n[n][:, k, :] if src is None
                            else src[:, k, n * NF:(n + 1) * NF])

                # fp8 j-blocks grouped contiguously (after the bf16 ones) so
                # the PE pays the fp8<->bf16 mode-switch cost once per layer,
                # not once per n-chunk
                loop = (([(j, n) for n in range(SN) for j in range(nj_bf)]
                         + [(j, n) for j in range(nj_bf, DT)
                            for n in range(SN)]) if first
                        else [(j, n) for j in range(DT) for n in range(SN)])
                for j, n in loop:
                    ps = pm.tile([P, NF], F32, tag="mlp")
                    if j < nj_bf:
                        for k in range(DT):
                            nc.tensor.matmul(
                                ps,
                                w_j[j][:, k, :],
                                rhs_sl(n, k),
                                start=(k == 0),
                                stop=(k == DT - 1),
                            )
                        scale = 1.0
                    else:
                        for k in range(0, DT, 2):
                            nc.tensor.matmul(
                                ps,
                                w8[:, j - nj_bf, k:k + 2, :],
                                xt8_sb[:, k:k + 2, n * NF:(n + 1) * NF],
                                start=(k == 0),
                                stop=(k == DT - 2),
                                perf_mode=mybir.MatmulPerfMode.DoubleRow,
                            )
                        scale = 1.0 / W8SCALE
                    dslice = (dst[n][:, j, :] if isinstance(dst, list)
                              else dst[:, j, n * NF:(n + 1) * NF])
                    nc.scalar.activation(out=dslice, in_=ps, func=AF.Tanh,
                                         scale=scale)

            def load_xn():
                xn_r = xn_d.rearrange("(t p) d -> p t d", p=P)
                for t in range(0, ST, 4):
                    nc.sync.dma_start(out=xn_sb[:, t:t + 4, :],
                                      in_=xn_r[:, t:t + 4, :])

            mlp_layer(None, wk1_d, h1_sb, first=True,
                      w8_dram=wk18_d if J8 else None)
            mlp_layer(h1_sb, wk2_d, kt_sb)
            mlp_layer(None, wq1_d, h1_sb, xdma=load_xn,
                      w8_dram=wq18_d if J8 else None)
            mlp_layer(h1_sb, wq2_d, qt_n)

        # ---- phase B: scores -> softmax -> transpose -> attended ----
        SB = 4   # super-block: 4 blocks of scores, then their 4 attendeds
        with tc.tile_pool(name="wexp", bufs=SB) as wexp_pool, \
             tc.tile_pool(name="wtT", bufs=SB + 1) as wtT_pool, \
             tc.tile_pool(name="sums", bufs=2 * SB + 2) as sums_pool, \
             tc.tile_pool(name="outst", bufs=2) as out_pool, \
             tc.tile_pool(name="psum_at", bufs=3, space="PSUM") as pat:

            def scores_softmax_transpose(i):
                """Row-block i of exp(scores) plus its reciprocal row sums,
                transposed into lhsT layout for the attended matmul."""
                wexp = wexp_pool.tile([P, S], BF, tag="wexp")
                sums = sums_pool.tile([P, SN], F32, tag="sums")
                for n in range(SN):
                    ps = psc.tile([P, NF], F32, tag="sc")
                    # fp8 DoubleRow: each matmul contracts a k-pair (K=256)
                    # at ~2 MACs/cell/cycle
                    for k in range(0, DT, 2):
                        nc.tensor.matmul(
                            ps,
                            kt_sb[:, k:k + 2, i * P:(i + 1) * P],
                            qt_n[n][:, k:k + 2, :],
                            start=(k == 0),
                            stop=(k == DT - 2),
                            perf_mode=mybir.MatmulPerfMode.DoubleRow,
                        )
                    # scores are bounded (|sc/32| < ~3): exp without max-shift
                    nc.scalar.activation(
                        out=wexp[:, n * NF:(n + 1) * NF],
                        in_=ps,
                        func=AF.Exp,
                        scale=float(SCALE),
                        accum_out=sums[:, n:n + 1],
                    )
                rcp = sums_pool.tile([P, 1], F32, tag="rcp")
                nc.vector.reduce_sum(rcp, sums, axis=AX)
                nc.vector.reciprocal(rcp, rcp)
                # one xbar transpose of the whole row-block:
                #   wtT[p, t, c] = wexp[c, t*128 + p]
                # transpose rides the SP ring: the ACT queue already carries
                # ~5us/block of exp+accum work during a scores burst (vs
                # 3.46us of PE time), and late exps stall the scores PSUM
                # bank rotation
                wtT = wtT_pool.tile([P, ST, P], BF, tag="wtT")
                nc.sync.dma_start_transpose(out=wtT, in_=wexp)
                return wtT, rcp

            def attended(i, wtT, rcp, last=False):
                outst = out_pool.tile([P, D], F32, tag="outst")
                # on the final block, shrink the trailing chunks so the
                # serial mul+DMA tail after the very last matmul is short;
                # tail chunks ride the (by then idle) ACT ring
                chunks = ([(0, NF), (NF, NF // 2), (3 * NF // 2, NF // 4),
                           (7 * NF // 4, NF // 4)]
                          if last else [(0, NF), (NF, NF)])
                # PSUM banks in phase B: psc 3 + at512 2 + at256 1 + at128 2 = 8
                for off, w in chunks:
                    ps = pat.tile([P, w], F32, tag=f"at{w}", name=f"at{w}",
                                  bufs=2 if w in (NF, NF // 4) else 1)
                    for t in range(ST):
                        nc.tensor.matmul(
                            ps,
                            wtT[:, t, :],
                            xn_sb[:, t, off:off + w],
                            start=(t == 0),
                            stop=(t == ST - 1),
                        )
                    # fold the softmax normalization into the PSUM->SBUF copy
                    nc.scalar.mul(outst[:, off:off + w], ps, rcp)
                    ring = nc.scalar if w < NF else nc.sync
                    ring.dma_start(
                        out=out_d[i * P:(i + 1) * P, off:off + w],
                        in_=outst[:, off:off + w],
                    )

            # super-blocked: SB blocks of fp8 scores back-to-back, then their
            # SB bf16 attendeds - fp8<->bf16 mode switches drop from 2/block
            # to 2/super-block, and each transpose has SB-1 score blocks plus
            # the attended queue as latency cover
            for a in range(0, ST, SB):
                cur = [scores_softmax_transpose(i) for i in range(a, a + SB)]
                for i in range(a, a + SB):
                    attended(i, *cur[i - a], last=(i == ST - 1))

    nc.compile()
    return nc


_NC = None


def _get_nc():
    global _NC
    if _NC is None:
        _NC = build_nc()
    return _NC


def _prep_w(w, dt=None, scale=1.0):
    """[d_out, d_in] f32 -> [p, j, k, c] of w.T (k,p index d_in; j,c d_out)."""
    wt = (np.asarray(w).T * scale).reshape(DT, P, DT, P).transpose(1, 2, 0, 3)
    return np.ascontiguousarray(wt).astype(dt or ml_dtypes.bfloat16)


def make_in_maps(sequence, Kw1, Kw2, Qw1, Qw2):
    bf16 = ml_dtypes.bfloat16
    seq = np.ascontiguousarray(np.transpose(np.asarray(sequence), (1, 0, 2)))  # [B, S, D]
    ws = {"wk1": _prep_w(Kw1), "wk2": _prep_w(Kw2),
          "wq1": _prep_w(Qw1), "wq2": _prep_w(Qw2)}
    if J8:
        e4 = ml_dtypes.float8_e4m3
        ws["wk18"] = np.ascontiguousarray(
            _prep_w(Kw1, e4, W8SCALE)[:, DT - J8:, :, :])
        ws["wq18"] = np.ascontiguousarray(
            _prep_w(Qw1, e4, W8SCALE)[:, DT - J8:, :, :])
    in_maps = []
    for b in range(B):
        xb = seq[b]
        xtf = np.ascontiguousarray(xb.T)                    # [D, S] fp32
        xc = np.ascontiguousarray(                          # [P, SN, DT, NF]
            xtf.reshape(DT, P, SN, NF).transpose(1, 2, 0, 3))
        m = {"xn": xb.astype(bf16), "xc": xc.astype(bf16)}
        if J8:
            m["xt8"] = xtf.astype(ml_dtypes.float8_e4m3)
        m.update(ws)
        in_maps.append(m)
    return in_maps


def kernel(sequence, Kw1, Kw2, Qw1, Qw2):
    nc = _get_nc()
    in_maps = make_in_maps(sequence, Kw1, Kw2, Qw1, Qw2)
    res = run_bass_kernel_spmd(nc, in_maps, core_ids=list(range(B)))
    out = np.stack([res.results[b]["out"] for b in range(B)], axis=1)
    return out.astype(np.float32)

